# revision 25
# baseline (speedup 1.0000x reference)
"""Trainium2 Bass kernel for nn_Encoder_71150428225961.

6-layer weight-shared transformer encoder (B=32, S=256, D=1024, H=16,
HID=4096, V=32000), data-parallel over batch across 8 NeuronCores
(4 sequences per core, weights replicated, no collectives).

Per-core layout strategy: the residual stream is kept TRANSPOSED
(x^T: [D on partitions, tokens on free dim], fp32) so every projection
matmul consumes operands with the contraction dim (d_model / d_hidden)
on partitions and needs no on-chip transposes.  LayerNorm statistics
(per-token, i.e. across partitions) are computed with one-hot matmuls
against a [sum; sum-of-squares] PSUM accumulator; softmax runs in
"scores-transposed" layout ([k_tok partitions, q_tok free]) so the
length mask folds into the Exp activation's per-partition bias and the
softmax denominator falls out of an appended ones-column on V.
Normalization scalars that vary along the free dim are replicated
across partitions with GPSIMD partition_broadcast.

Weights are cast to bf16 (PE runs 1 cycle/row vs 2 for fp32); PSUM
accumulation stays fp32, the residual stream stays fp32.
"""

import math
from contextlib import ExitStack

import numpy as np
import ml_dtypes

import concourse.bass as bass
import concourse.mybir as mybir
import concourse.tile as tile
from concourse import bacc

f32 = mybir.dt.float32
bf16 = mybir.dt.bfloat16
i32 = mybir.dt.int32
AF = mybir.ActivationFunctionType
OP = mybir.AluOpType

P = 128
B, S, D, H, HID, V, L = 32, 256, 1024, 16, 4096, 32000, 6
DH = D // H          # 64
ND = D // P          # 8   d-model partition tiles
NHT = HID // P       # 32  hidden partition tiles
NCORES = 8
BL = B // NCORES     # 4 sequences per core
T = BL * S           # 1024 tokens per core
HALF = T // 2        # 512-token halves (2 sequences)
QT = 256             # token quarter (one sequence)
SCALE = 1.0 / math.sqrt(DH)
NEG = -30000.0       # additive mask value (clamps exp to 0, no overflow)
EPS = 1e-5


def _scaled_identity(nc, tile_ap, scale):
    nc.gpsimd.memset(tile_ap, 0.0)
    nc.gpsimd.affine_select(
        out=tile_ap, in_=tile_ap, compare_op=OP.not_equal,
        fill=float(scale), base=0, pattern=[[-1, P]], channel_multiplier=1,
    )


def build_program(flags):
    n_layers = flags["n_layers"]
    hw_loop = flags["hw_loop"]

    nc = bacc.Bacc("TRN2", target_bir_lowering=False, debug=False,
                   num_devices=NCORES)

    # ---------------- DRAM tensors ----------------
    tab = nc.dram_tensor("tab", [T, D], f32, kind="ExternalInput")
    idx = nc.dram_tensor("idx", [P, T // P], i32, kind="ExternalInput")
    peT = nc.dram_tensor("peT", [D, S], f32, kind="ExternalInput")
    amask = nc.dram_tensor("amask", [P, BL * 2], f32, kind="ExternalInput")
    wq_d = nc.dram_tensor("wq", [D, D], bf16, kind="ExternalInput")
    wk_d = nc.dram_tensor("wk", [D, D], bf16, kind="ExternalInput")
    wv_d = nc.dram_tensor("wv", [D, D], bf16, kind="ExternalInput")
    wo_d = nc.dram_tensor("wo", [D, D], bf16, kind="ExternalInput")
    # W1/W2 pre-repacked on host so each streamed chunk is one contiguous
    # run per partition (chunked DMA at full efficiency).
    w1_d = nc.dram_tensor("w1", [16, P, ND, 256], bf16, kind="ExternalInput")
    w2_d = nc.dram_tensor("w2", [2, 16, P, 2, 512], bf16,
                          kind="ExternalInput")
    if flags["use_bq"]:
        bq_d = nc.dram_tensor("bq", [P, ND], f32, kind="ExternalInput")
    if flags["use_bk"]:
        bk_d = nc.dram_tensor("bk", [P, ND], f32, kind="ExternalInput")
    if flags["use_bv"]:
        bv_d = nc.dram_tensor("bv", [1, D], f32, kind="ExternalInput")
    if flags["use_bo"]:
        bo_d = nc.dram_tensor("bo", [P, ND], f32, kind="ExternalInput")
    if flags["use_b1"]:
        b1_d = nc.dram_tensor("b1", [P, NHT], f32, kind="ExternalInput")
    if flags["use_b2"]:
        b2_d = nc.dram_tensor("b2", [P, ND], f32, kind="ExternalInput")
    if flags["ln_a_aff"]:
        lnas_d = nc.dram_tensor("lnas", [P, ND], f32, kind="ExternalInput")
        lnab_d = nc.dram_tensor("lnab", [P, ND], f32, kind="ExternalInput")
    if flags["ln_f_aff"]:
        lnfs_d = nc.dram_tensor("lnfs", [P, ND], f32, kind="ExternalInput")
        lnfb_d = nc.dram_tensor("lnfb", [P, ND], f32, kind="ExternalInput")
    if flags["ln_o_aff"]:
        lnos_d = nc.dram_tensor("lnos", [P, ND], f32, kind="ExternalInput")
        lnob_d = nc.dram_tensor("lnob", [P, ND], f32, kind="ExternalInput")
    out_d = nc.dram_tensor("out", [T, D], f32, kind="ExternalOutput")


    with tile.TileContext(nc) as tc, ExitStack() as ctx:
        wp = ctx.enter_context(tc.tile_pool(name="wp", bufs=1))
        sp = ctx.enter_context(tc.tile_pool(name="sp", bufs=2))
        xp = ctx.enter_context(tc.tile_pool(name="xp", bufs=1))
        ap = ctx.enter_context(tc.tile_pool(name="ap", bufs=1))
        hp = ctx.enter_context(tc.tile_pool(name="hp", bufs=1))
        ep = ctx.enter_context(tc.tile_pool(name="ep", bufs=2))
        rp = ctx.enter_context(tc.tile_pool(name="rp", bufs=2))
        lp = ctx.enter_context(tc.tile_pool(name="lp", bufs=2))
        st = ctx.enter_context(tc.tile_pool(name="st", bufs=4))
        tp = ctx.enter_context(tc.tile_pool(name="tp", bufs=2))
        pp = ctx.enter_context(tc.tile_pool(name="pp", bufs=4, space="PSUM"))
        fp = ctx.enter_context(tc.tile_pool(name="fp", bufs=4, space="PSUM"))

        # ---------------- constants / resident weights ----------------
        wq_sb = wp.tile([P, ND, D], bf16)
        nc.sync.dma_start(wq_sb, wq_d.ap().rearrange("(c p) n -> p c n", p=P))
        wk_sb = wp.tile([P, ND, D], bf16)
        nc.sync.dma_start(wk_sb, wk_d.ap().rearrange("(c p) n -> p c n", p=P))
        wv_sb = wp.tile([P, ND, D], bf16)
        nc.sync.dma_start(wv_sb, wv_d.ap().rearrange("(c p) n -> p c n", p=P))
        wo_sb = wp.tile([P, ND, D], bf16)
        nc.sync.dma_start(wo_sb, wo_d.ap().rearrange("(c p) n -> p c n", p=P))
        amask_sb = wp.tile([P, BL * 2], f32)
        nc.sync.dma_start(amask_sb, amask.ap())
        idx_sb = wp.tile([P, T // P], i32)
        nc.sync.dma_start(idx_sb, idx.ap())

        ones_f = wp.tile([P, 1], f32)   # ones column (fp32 sum matmuls)
        nc.vector.memset(ones_f, 1.0)
        id32 = wp.tile([P, P], f32)     # sqrt(D) * I for embedding transpose
        _scaled_identity(nc, id32, math.sqrt(D))
        id1 = wp.tile([P, P], f32)
        _scaled_identity(nc, id1, 1.0)
        eps_t = wp.tile([1, 1], f32)
        nc.vector.memset(eps_t, EPS)

        def load_pp(dram):
            t = wp.tile([P, ND], f32, name=dram.name + "_sb")
            nc.sync.dma_start(t, dram.ap())
            return t

        bq_sb = load_pp(bq_d) if flags["use_bq"] else None
        bk_sb = load_pp(bk_d) if flags["use_bk"] else None
        bo_sb = load_pp(bo_d) if flags["use_bo"] else None
        b2_sb = load_pp(b2_d) if flags["use_b2"] else None
        lnas_sb = load_pp(lnas_d) if flags["ln_a_aff"] else None
        lnab_sb = load_pp(lnab_d) if flags["ln_a_aff"] else None
        lnfs_sb = load_pp(lnfs_d) if flags["ln_f_aff"] else None
        lnfb_sb = load_pp(lnfb_d) if flags["ln_f_aff"] else None
        lnos_sb = load_pp(lnos_d) if flags["ln_o_aff"] else None
        lnob_sb = load_pp(lnob_d) if flags["ln_o_aff"] else None
        if flags["use_b1"]:
            b1_sb = wp.tile([P, NHT], f32)
            nc.sync.dma_start(b1_sb, b1_d.ap())
        if flags["use_bv"]:
            bv_bc = wp.tile([P, D], bf16)
            nc.sync.dma_start(
                bv_bc,
                bass.AP(tensor=bv_d.ap().tensor, offset=0, ap=[[0, P], [1, D]]),
            )

        # persistent per-half activations (single-buffered; x double-buffered)
        x_res = ap.tile([P, ND, T], f32)      # resident residual stream
        q_t = ap.tile([P, ND, HALF], bf16)
        k_t = ap.tile([P, ND, HALF], bf16)
        vpad = ap.tile([P, 4, H, DH], bf16)   # v in natural layout
        ones_b = wp.tile([P, 1], bf16)
        nc.vector.memset(ones_b, 1.0)

        # ---------------- embedding: x0 = emb[src]*sqrt(D) + pe ----------------
        for tt in range(8):
            x0 = tp.tile([P, D], f32, tag="tmp", bufs=1)
            nc.gpsimd.indirect_dma_start(
                out=x0, out_offset=None, in_=tab.ap(),
                in_offset=bass.IndirectOffsetOnAxis(
                    ap=idx_sb[:, tt:tt + 1], axis=0),
            )
            for dc in range(ND):
                tps = pp.tile([P, P], f32, tag="a")
                # regular matmul vs sqrt(D)*I: scaled transpose
                nc.tensor.matmul(tps, x0[:, dc * P:(dc + 1) * P], id32,
                                 start=True, stop=True)
                pe_c = tp.tile([P, P], f32, tag="oc", bufs=1, name="pe_c")
                nc.sync.dma_start(
                    pe_c, peT.ap()[dc * P:(dc + 1) * P,
                                   (tt % 2) * P:((tt % 2) + 1) * P])
                nc.vector.tensor_tensor(
                    out=x_res[:, dc, tt * P:(tt + 1) * P], in0=tps,
                    in1=pe_c, op=OP.add)

        # ---------------- LayerNorm helper (one token-half) ----------------
        def emit_ln(x_t, sa_sb, sb_sb, out16, fin=False):
            """x_t: [P, ND, HALF] f32.  Writes out16 (bf16 [P,ND,HALF]) or
            returns list of f32 [P,HALF] tiles when fin=True."""
            sums_x = pp.tile([1, HALF], f32, tag="a")
            sums_q = pp.tile([1, HALF], f32, tag="a")
            for dc in range(ND):
                xsq = tp.tile([P, HALF], bf16, tag="xsq", bufs=2)
                nc.scalar.activation(xsq, x_t[:, dc, :], AF.Square)
                nc.tensor.matmul(sums_x, ones_f, x_t[:, dc, :],
                                 start=(dc == 0), stop=(dc == ND - 1))
                nc.tensor.matmul(sums_q, ones_b, xsq,
                                 start=(dc == 0), stop=(dc == ND - 1))
            m = st.tile([1, HALF], f32, tag="st")
            nc.scalar.mul(m, sums_x, 1.0 / D)
            msq = st.tile([1, HALF], f32, tag="st")
            nc.scalar.mul(msq, sums_q, 1.0 / D)
            m2 = st.tile([1, HALF], f32, tag="st")
            nc.vector.tensor_tensor(out=m2, in0=m, in1=m, op=OP.mult)
            var = st.tile([1, HALF], f32, tag="st")
            nc.vector.tensor_tensor(out=var, in0=msq, in1=m2,
                                    op=OP.subtract)
            sd = st.tile([1, HALF], f32, tag="st")
            nc.scalar.activation(sd, var, AF.Sqrt, bias=eps_t[0:1, 0:1])
            r = st.tile([1, HALF], f32, tag="st")
            nc.vector.reciprocal(r, sd)
            mr = st.tile([1, HALF], f32, tag="st")
            nc.vector.tensor_tensor(out=mr, in0=m, in1=r, op=OP.mult)
            rb = lp.tile([P, HALF], f32, tag="rb")
            nc.gpsimd.partition_broadcast(rb, r)
            outs = []
            if fin:
                mrb = lp.tile([P, HALF], f32, tag="rb", name="mrbf")
                nc.gpsimd.partition_broadcast(mrb, mr)
                for dc in range(ND):
                    xf = tp.tile([P, HALF], f32, tag="tmp", bufs=1)
                    nc.vector.tensor_tensor(out=xf, in0=x_t[:, dc, :], in1=rb,
                                            op=OP.mult)
                    nc.vector.tensor_tensor(out=xf, in0=xf, in1=mrb,
                                            op=OP.subtract)
                    if sa_sb is not None:
                        nc.vector.tensor_scalar(
                            out=xf, in0=xf,
                            scalar1=sa_sb[:, dc:dc + 1],
                            scalar2=sb_sb[:, dc:dc + 1],
                            op0=OP.mult, op1=OP.add)
                    outs.append(xf)
                return outs
            mr16 = st.tile([1, HALF], bf16, tag="st16", bufs=2)
            nc.vector.tensor_copy(mr16, mr)
            mrb = lp.tile([P, HALF], bf16, tag="mrb")
            nc.gpsimd.partition_broadcast(mrb, mr16)
            for dc in range(ND):
                nc.vector.tensor_tensor(out=out16[:, dc, :],
                                        in0=x_t[:, dc, :], in1=rb, op=OP.mult)
                nc.vector.tensor_tensor(out=out16[:, dc, :],
                                        in0=out16[:, dc, :], in1=mrb,
                                        op=OP.subtract)
                if sa_sb is not None:
                    nc.vector.tensor_scalar(
                        out=out16[:, dc, :], in0=out16[:, dc, :],
                        scalar1=sa_sb[:, dc:dc + 1],
                        scalar2=sb_sb[:, dc:dc + 1],
                        op0=OP.mult, op1=OP.add)
            return None

        # ---------------- one layer for one token-half ----------------
        stage = flags.get("stage", 99)
        est = 4 if stage == 40 else stage

        def layer_half(half):
            x_t = x_res[:, :, half * HALF:(half + 1) * HALF]
            xn_t = xp.tile([P, ND, HALF], bf16, tag="xn", bufs=2, name="xn_t")

            # --- LN-a -> xn ---
            if est >= 1:
                emit_ln(x_t, lnas_sb, lnab_sb, xn_t)

            # --- Q, K projections (out transposed: [dq, tok]) ---
            for which, w_sb, b_sb, dst_t in () if est < 2 else (
                ("q", wq_sb, bq_sb, q_t), ("k", wk_sb, bk_sb, k_t),
            ):
                for ds_ in range(ND):
                    ps_ = pp.tile([P, HALF], f32, tag="a", name="qk_ps")
                    for dci in range(ND):
                        nc.tensor.matmul(
                            ps_, w_sb[:, dci, ds_ * P:(ds_ + 1) * P],
                            xn_t[:, dci, :],
                            start=(dci == 0), stop=(dci == ND - 1))
                    if b_sb is not None:
                        nc.vector.tensor_scalar_add(
                            out=dst_t[:, ds_, :], in0=ps_,
                            scalar1=b_sb[:, ds_:ds_ + 1])
                    else:
                        nc.vector.tensor_copy(dst_t[:, ds_, :], ps_)

            # --- V projection (natural layout: [tok, dv] into vpad) ---
            for ttl in range(4) if est >= 3 else []:
                tcol = ttl * P
                for vc in range(2):
                    vps = pp.tile([P, 8, DH], f32, tag="a", name="v_ps")
                    for dci in range(ND):
                        nc.tensor.matmul(
                            vps, xn_t[:, dci, tcol:tcol + P],
                            wv_sb[:, dci, vc * 512:(vc + 1) * 512],
                            start=(dci == 0), stop=(dci == ND - 1))
                    dst = vpad[:, ttl, vc * 8:(vc + 1) * 8, :]
                    if flags["use_bv"]:
                        bvv = bv_bc[:, vc * 512:(vc + 1) * 512]
                        nc.vector.tensor_tensor(
                            out=dst, in0=vps,
                            in1=bvv.rearrange("p (h d) -> p h d", h=8),
                            op=OP.add)
                    else:
                        nc.vector.tensor_copy(dst, vps)

            # --- attention (o^T written into xn_t buffer) ---
            # scores/e in [k_tok partitions, q_tok free] layout, heads in
            # pairs (even head -> psum rows 0:64, odd head -> 64:128 via
            # tile_position col packing).  e is normalized by the softmax
            # denominator BEFORE p@v so everything stays partition-aligned.
            o_t = xn_t   # o^T overwrites the LN-a output in place
            for b in range(2) if est >= 4 else []:
                bg = half * 2 + b
                col = b * S
                for h in range(H):
                    po = (h % 2) * DH
                    dc = h // 2
                    sps = pp.tile([P, 2, S], f32, tag="a", name="s_ps")
                    for kt in range(2):
                        nc.tensor.matmul(
                            sps[:, kt, :],
                            k_t[po:po + DH, dc, col + kt * P:col + kt * P + P],
                            q_t[po:po + DH, dc, col:col + S],
                            start=True, stop=True)
                    et = ep.tile([P, 2, S], bf16, tag="e")
                    for kt in range(2):
                        nc.scalar.activation(
                            et[:, kt, :], sps[:, kt, :], AF.Exp,
                            bias=amask_sb[:, bg * 2 + kt:bg * 2 + kt + 1],
                            scale=SCALE)
                    rbt = rp.tile([P, S], bf16, tag="rb")
                    if stage == 40:
                        nc.vector.memset(rbt, 1.0)
                    else:
                        ssum = fp.tile([1, S], f32, tag="f", name="ss_ps")
                        for kt in range(2):
                            nc.tensor.matmul(ssum, ones_b, et[:, kt, :],
                                             start=(kt == 0), stop=(kt == 1))
                        rr = st.tile([1, S], bf16, tag="r", bufs=2)
                        with nc.allow_low_precision("softmax denom in bf16"):
                            nc.vector.reciprocal(rr, ssum)
                        nc.gpsimd.partition_broadcast(rbt, rr)
                    ups = fp.tile([P, S], f32, tag="f", name="u_ps")
                    for kt in range(2):
                        nc.tensor.matmul(
                            ups[po:po + DH, :],
                            vpad[:, b * 2 + kt, h, :], et[:, kt, :],
                            start=(kt == 0), stop=(kt == 1),
                            tile_position=(0, po))
                    # o = (p@v unnormalized) * 1/denom, fused into the copy
                    nc.vector.tensor_tensor(
                        out=o_t[po:po + DH, dc, col:col + S],
                        in0=ups[po:po + DH, :], in1=rbt[po:po + DH, :],
                        op=OP.mult)

            # --- out projection + residual into x ---
            for ds_ in range(ND) if est >= 5 else []:
                ops_ = pp.tile([P, HALF], f32, tag="a", name="o_ps")
                for dci in range(ND):
                    nc.tensor.matmul(
                        ops_, wo_sb[:, dci, ds_ * P:(ds_ + 1) * P],
                        o_t[:, dci, :],
                        start=(dci == 0), stop=(dci == ND - 1))
                nc.vector.tensor_tensor(out=x_t[:, ds_, :],
                                        in0=x_t[:, ds_, :], in1=ops_,
                                        op=OP.add)
                if bo_sb is not None:
                    nc.vector.tensor_scalar_add(
                        out=x_t[:, ds_, :], in0=x_t[:, ds_, :],
                        scalar1=bo_sb[:, ds_:ds_ + 1])

            # --- LN-f -> new xn tile ---
            xn_t = xp.tile([P, ND, HALF], bf16, tag="xn", bufs=2, name="xn_f")
            if est >= 6:
                emit_ln(x_t, lnfs_sb, lnfb_sb, xn_t)

            # --- FFN over the full token-half ---
            # W1 phase materializes h for the half (32KB); W2 runs two
            # dout-passes of 4 accumulation chains, one chain per psum
            # bank (start=True clears has_written bank-wide).  Streaming
            # W1/W2 once per half (not per quarter) halves HBM traffic.
            for _ffn in range(1) if est >= 7 else []:
                h_h = hp.tile([P, NHT, HALF], bf16, tag="h")
                for e in range(16):
                    w1s = sp.tile([P, ND, 2 * P], bf16, tag="w1")
                    nc.sync.dma_start(w1s, w1_d.ap()[e])
                    for hh in range(2):
                        ht_i = e * 2 + hh
                        hps = pp.tile([P, HALF], f32, tag="a", name="h_ps")
                        for dci in range(ND):
                            nc.tensor.matmul(
                                hps, w1s[:, dci, hh * P:(hh + 1) * P],
                                xn_t[:, dci, :],
                                start=(dci == 0), stop=(dci == ND - 1))
                        if flags["use_b1"]:
                            nc.scalar.activation(
                                h_h[:, ht_i, :], hps, AF.Gelu,
                                bias=b1_sb[:, ht_i:ht_i + 1])
                        else:
                            nc.scalar.activation(h_h[:, ht_i, :], hps,
                                                 AF.Gelu)
                for dpass in range(2):
                    fps = [fp.tile([P, HALF], f32, tag="f", name="f_ps")
                           for _ in range(4)]
                    for e in range(16):
                        w2s = sp.tile([P, 2, 512], bf16, tag="w2")
                        nc.scalar.dma_start(w2s, w2_d.ap()[dpass, e])
                        for dd in range(4):
                            for hh in range(2):
                                nc.tensor.matmul(
                                    fps[dd],
                                    w2s[:, hh, dd * P:(dd + 1) * P],
                                    h_h[:, e * 2 + hh, :],
                                    start=(e == 0 and hh == 0),
                                    stop=(e == 15 and hh == 1))
                    for dd in range(4):
                        do_ = dpass * 4 + dd
                        nc.vector.tensor_tensor(
                            out=x_t[:, do_, :], in0=x_t[:, do_, :],
                            in1=fps[dd], op=OP.add)
                        if b2_sb is not None:
                            nc.vector.tensor_scalar_add(
                                out=x_t[:, do_, :], in0=x_t[:, do_, :],
                                scalar1=b2_sb[:, do_:do_ + 1])

        def layer_body():
            for half in range(2):
                layer_half(half)

        if hw_loop and n_layers > 1:
            with tc.For_i(0, n_layers, 1) as _i:
                layer_body()
        else:
            for _ in range(n_layers):
                layer_body()

        # ---------------- final LN + transpose to natural + store ----------------
        for half in range(2):
            x_t = x_res[:, :, half * HALF:(half + 1) * HALF]
            xfs = emit_ln(x_t, lnos_sb, lnob_sb, None, fin=True)
            for dc in range(ND):
                for ttl in range(4):
                    tps = pp.tile([P, P], f32, tag="a", name="t_ps")
                    nc.tensor.matmul(tps, xfs[dc][:, ttl * P:(ttl + 1) * P],
                                     id1, start=True, stop=True)
                    oc = tp.tile([P, P], f32, tag="oc", bufs=1)
                    nc.vector.tensor_copy(oc, tps)
                    row = half * HALF + ttl * P
                    nc.sync.dma_start(
                        out_d.ap()[row:row + P, dc * P:(dc + 1) * P], oc)

    nc.compile()
    return nc


# ------------------------------------------------------------------
# host side
# ------------------------------------------------------------------
_PROG_CACHE = {}


def _get_program(flags):
    key = tuple(sorted(flags.items()))
    if key not in _PROG_CACHE:
        _PROG_CACHE[key] = build_program(dict(flags))
    return _PROG_CACHE[key]


def _pp_layout(v):
    """[D] vector -> [P, D//P] per-partition layout (d = dc*128 + p)."""
    return np.ascontiguousarray(v.reshape(-1, P).T).astype(np.float32)


def _pos_encoding_np():
    pos = np.arange(S, dtype=np.float32)[:, None]
    freq = np.exp(np.arange(0, D, 2, dtype=np.float32)
                  * (-math.log(10000.0) / D))[None, :]
    arg = pos * freq
    pe = np.zeros((S, D), np.float32)
    pe[:, 0::2] = np.sin(arg)
    pe[:, 1::2] = np.cos(arg)
    return pe


def prep(inputs, n_layers=L, hw_loop=True):
    """Returns (flags, in_maps) for the 8 cores."""
    src = np.asarray(inputs["source"]).astype(np.int64)
    lens = np.asarray(inputs["source_lengths"]).astype(np.int64)
    emb = np.asarray(inputs["emb"], dtype=np.float32)
    Wq = np.asarray(inputs["Wq"], np.float32)
    bq = np.asarray(inputs["bq"], np.float32)
    Wkv = np.asarray(inputs["Wkv"], np.float32)
    bkv = np.asarray(inputs["bkv"], np.float32)
    Wo = np.asarray(inputs["Wo"], np.float32)
    bo = np.asarray(inputs["bo"], np.float32)
    W1 = np.asarray(inputs["W1"], np.float32)
    b1 = np.asarray(inputs["b1"], np.float32)
    W2 = np.asarray(inputs["W2"], np.float32)
    b2 = np.asarray(inputs["b2"], np.float32)
    ln = {k: np.asarray(inputs[k], np.float32) for k in
          ("ln_a_s", "ln_a_b", "ln_f_s", "ln_f_b", "ln_out_s", "ln_out_b")}

    flags = dict(
        n_layers=n_layers, hw_loop=hw_loop,
        use_bq=bool(np.any(bq != 0)), use_bk=bool(np.any(bkv[:D] != 0)),
        use_bv=bool(np.any(bkv[D:] != 0)), use_bo=bool(np.any(bo != 0)),
        use_b1=bool(np.any(b1 != 0)), use_b2=bool(np.any(b2 != 0)),
        ln_a_aff=not (np.all(ln["ln_a_s"] == 1) and np.all(ln["ln_a_b"] == 0)),
        ln_f_aff=not (np.all(ln["ln_f_s"] == 1) and np.all(ln["ln_f_b"] == 0)),
        ln_o_aff=not (np.all(ln["ln_out_s"] == 1)
                      and np.all(ln["ln_out_b"] == 0)),
    )

    shared = {
        "peT": np.ascontiguousarray(_pos_encoding_np().T),
        "wq": Wq.astype(ml_dtypes.bfloat16),
        "wk": np.ascontiguousarray(Wkv[:, :D]).astype(ml_dtypes.bfloat16),
        "wv": np.ascontiguousarray(Wkv[:, D:]).astype(ml_dtypes.bfloat16),
        "wo": Wo.astype(ml_dtypes.bfloat16),
        "w1": np.ascontiguousarray(
            W1.reshape(ND, P, 16, 256).transpose(2, 1, 0, 3)
        ).astype(ml_dtypes.bfloat16),
        "w2": np.ascontiguousarray(
            W2.reshape(16, 2, P, 2, 512).transpose(3, 0, 2, 1, 4)
        ).astype(ml_dtypes.bfloat16),
    }
    if flags["use_bq"]:
        shared["bq"] = _pp_layout(bq)
    if flags["use_bk"]:
        shared["bk"] = _pp_layout(bkv[:D])
    if flags["use_bv"]:
        shared["bv"] = bkv[D:].reshape(1, D).astype(np.float32)
    if flags["use_bo"]:
        shared["bo"] = _pp_layout(bo)
    if flags["use_b1"]:
        shared["b1"] = _pp_layout(b1)
    if flags["use_b2"]:
        shared["b2"] = _pp_layout(b2)
    if flags["ln_a_aff"]:
        shared["lnas"] = _pp_layout(ln["ln_a_s"])
        shared["lnab"] = _pp_layout(ln["ln_a_b"])
    if flags["ln_f_aff"]:
        shared["lnfs"] = _pp_layout(ln["ln_f_s"])
        shared["lnfb"] = _pp_layout(ln["ln_f_b"])
    if flags["ln_o_aff"]:
        shared["lnos"] = _pp_layout(ln["ln_out_s"])
        shared["lnob"] = _pp_layout(ln["ln_out_b"])

    in_maps = []
    kpos = np.arange(S)
    for c in range(NCORES):
        toks = src[c * BL:(c + 1) * BL].reshape(-1)      # [1024]
        uniq, inv = np.unique(toks, return_inverse=True)
        tab = np.zeros((T, D), np.float32)
        tab[:len(uniq)] = emb[uniq]
        idxs = inv.astype(np.int32).reshape(T // P, P).T  # [128, 8]
        am = np.zeros((P, BL * 2), np.float32)
        for b in range(BL):
            valid = kpos < lens[c * BL + b]               # [256]
            vv = np.where(valid, 0.0, NEG).astype(np.float32)
            am[:, b * 2] = vv[:P]
            am[:, b * 2 + 1] = vv[P:]
        m = dict(shared)
        m["tab"] = tab
        m["idx"] = np.ascontiguousarray(idxs)
        m["amask"] = am
        in_maps.append(m)
    return flags, in_maps


class Runner:
    """Cached PJRT executor for a compiled Bass program on 8 cores.

    Same mechanism run_bass_kernel_spmd uses under axon
    (bass2jax.run_bass_via_pjrt), but the jitted shard_map executable is
    built once and reused, so repeat calls skip recompilation and can be
    timed back-to-back.
    """

    def __init__(self, nc, n_cores=NCORES):
        import jax
        import concourse.mybir as _mybir
        from concourse.bass2jax import (
            _bass_exec_p, install_neuronx_cc_hook, partition_id_tensor,
        )
        from jax.sharding import Mesh, PartitionSpec
        from jax.experimental.shard_map import shard_map

        install_neuronx_cc_hook()
        assert nc.dbg_addr is None
        part_name = (nc.partition_id_tensor.name
                     if nc.partition_id_tensor else None)
        self.jax = jax
        self.n_cores = n_cores
        in_names, out_names, out_avals, zero_outs = [], [], [], []
        for alloc in nc.m.functions[0].allocations:
            if not isinstance(alloc, _mybir.MemoryLocationSet):
                continue
            name = alloc.memorylocations[0].name
            if alloc.kind == "ExternalInput":
                if name != part_name:
                    in_names.append(name)
            elif alloc.kind == "ExternalOutput":
                out_names.append(name)
                shape = tuple(alloc.tensor_shape)
                dtype = _mybir.dt.np(alloc.dtype)
                out_avals.append(jax.core.ShapedArray(shape, dtype))
                zero_outs.append(np.zeros(shape, dtype))
        self.in_names = list(in_names)
        self.out_names = out_names
        self.out_avals = out_avals
        self.zero_outs = zero_outs
        n_params = len(in_names)
        n_outs = len(out_names)
        all_names = in_names + out_names
        if part_name is not None:
            all_names = all_names + [part_name]

        def _body(*args):
            operands = list(args)
            if part_name is not None:
                operands.append(partition_id_tensor())
            outs = _bass_exec_p.bind(
                *operands,
                out_avals=tuple(out_avals),
                in_names=tuple(all_names),
                out_names=tuple(out_names),
                lowering_input_output_aliases=(),
                sim_require_finite=True,
                sim_require_nnan=True,
                nc=nc,
            )
            return tuple(outs)

        devices = jax.devices()[:n_cores]
        assert len(devices) == n_cores
        self.mesh = Mesh(np.asarray(devices), ("core",))
        self.pspec = PartitionSpec("core")
        in_specs = (self.pspec,) * (n_params + n_outs)
        out_specs = (self.pspec,) * n_outs
        donate = tuple(range(n_params, n_params + n_outs))
        self.fn = jax.jit(
            shard_map(_body, mesh=self.mesh, in_specs=in_specs,
                      out_specs=out_specs, check_rep=False),
            donate_argnums=donate, keep_unused=True)

    def concat_inputs(self, in_maps):
        return [
            np.concatenate([np.asarray(in_maps[c][n])
                            for c in range(self.n_cores)], axis=0)
            for n in self.in_names
        ]

    def _zeros(self):
        return [np.zeros((self.n_cores * z.shape[0], *z.shape[1:]), z.dtype)
                for z in self.zero_outs]

    def __call__(self, concat_in):
        out_arrs = self.fn(*concat_in, *self._zeros())
        res = []
        for c in range(self.n_cores):
            res.append({
                n: np.asarray(out_arrs[i]).reshape(
                    self.n_cores, *self.out_avals[i].shape)[c]
                for i, n in enumerate(self.out_names)
            })
        return res

    def run(self, in_maps):
        return self(self.concat_inputs(in_maps))


_RUNNER_CACHE = {}


def get_runner(flags):
    key = tuple(sorted(flags.items()))
    if key not in _RUNNER_CACHE:
        _RUNNER_CACHE[key] = Runner(_get_program(flags))
    return _RUNNER_CACHE[key]


def run(nc, in_maps):
    from concourse.bass_utils import run_bass_kernel_spmd
    res = run_bass_kernel_spmd(nc, in_maps, core_ids=list(range(NCORES)),
                               trace=False)
    return res.results


def assemble(results):
    outs = [results[c]["out"].reshape(BL, S, D) for c in range(NCORES)]
    return np.concatenate(outs, axis=0).astype(np.float32)


def kernel(**inputs):
    flags, in_maps = prep(inputs)
    runner = get_runner(flags)
    return assemble(runner.run(in_maps))
